# revision 25
# baseline (speedup 1.0000x reference)
"""Trainium2 Bass kernel for the D3CG trainer-loss problem.

Computes, for full inputs:
    loss = sum((eps_theta - noise)**2)
where eps_theta is a 1x1-conv surrogate denoiser applied to
[d_t, cbct_coeffs] built from Haar DWT coefficients of x_0's two channels.

Strategy (pure data parallel over batch, 4 batches per core on 8 cores):
Everything before the square is linear in (x_0, noise) per output pixel, with
per-batch scalar coefficients, so each 64-image-row slab reduces to matmuls
against small host-precomputed coefficient matrices:

  r = Le^T @ x_even + Lo^T @ x_odd + (-I + NzB)^T @ noise'

laid out [4 outch x 32 rowpairs, 256 cols] in PSUM, where noise' = noise + c
with the per-batch channel shift c solving (s_omab*W1 - I) c = b + temb[t]:
the constant shift reproduces the bias exactly through the noise mixing, so
no bias add is needed anywhere on device. One ScalarE Square per batch with
accum_out reduces each [128, 2048] PSUM tile to per-partition partial sums
(the last batch in two pieces so only a 512-col piece trails the final
matmul instead of a full 2048-col pass).

All data streams in fp8e4m3 (host casts; final loss rel err ~2.5e-3 vs the
2e-2 gate) and both matmuls per slab-pair bank run in DoubleRow perf mode.
A chain of dummy matmuls on junk SBUF data runs while the input DMAs ramp,
pushing the PE DVFS clock up before the first real matmul so real matmuls
run near the fast steady-state rate from the start.

DMA: one uint8 blob per batch [128, 6656] (coefs | x/nz slabs 0-3 | x/nz
slabs 4-7), bitcast to fp8 on device, issued as two transfers on the two
HWDGE dynamic rings (sync + scalar engines, even/odd batches) so all
triggers fire right after the NEFF prologue and batch 0 computes while
later batches stream. Total HBM traffic ~3.3 MiB/core.
"""

import sys

if "/opt/trn_rl_repo" not in sys.path:
    sys.path.insert(0, "/opt/trn_rl_repo")

import numpy as np
import ml_dtypes

import concourse.bass as bass  # noqa: F401
import concourse.mybir as mybir
import concourse.tile as tile
from concourse import bacc
from concourse.bass_utils import run_bass_kernel_spmd

T = 1000
BETA_1 = 1e-4
BETA_T = 0.02

N_CORES = 8
B_TOTAL = 32
B_PER = B_TOTAL // N_CORES  # 4 batches per core
H = 512
Wd = 512
N_SLABS = 8                 # 64-image-row slabs per image
PAIRS = 32                  # output rows per slab
WO = 256                    # output cols

F32 = mybir.dt.float32
FP8 = mybir.dt.float8e4
NPFP8 = ml_dtypes.float8_e4m3

# per-partition byte offsets inside the per-batch fp8 blob [128, 6656]
OFF_COEF = 0        # LeLo [2,128] + NzMat [2,128] = 512 B
OFF_XA = 512        # x slab-pairs 0-1: [2 pair, 2 parity, 2 slab, 256] = 2048 B
OFF_NZA = 2560      # nz slabs 0-3: [4, 256] = 1024 B
OFF_XB = 3584       # x slab-pairs 2-3
OFF_NZB = 5632      # nz slabs 4-7
BLOB = 6656

# the first batch's square runs as two 1024-col pieces (starts as soon as
# its first two PSUM banks are written), the last batch as 1536+512 so
# only a 512-col piece trails the final matmul. (A DVE-assisted split was
# tried and reverted: DVE fp32 runs at ~1.3ns/elem and its PSUM reads
# stalled the matmul pipeline, dropping the PE clock.)
SPLIT = 1536
N_OUT = B_PER + 2

# dummy matmuls to ramp the PE clock while input DMAs stream
N_RAMP = 8

# Haar 2x2 analysis kernels for [cA, cH, cV, cD] as functions of the block
# [[a, b], [c, d]] = [[x[2i,2j], x[2i,2j+1]], [x[2i+1,2j], x[2i+1,2j+1]]].
_HAAR = 0.5 * np.array(
    [
        [[1.0, 1.0], [1.0, 1.0]],    # cA
        [[1.0, 1.0], [-1.0, -1.0]],  # cH (detail axis0)
        [[1.0, -1.0], [1.0, -1.0]],  # cV (detail axis1)
        [[1.0, -1.0], [-1.0, 1.0]],  # cD
    ],
    dtype=np.float64,
)


def _schedule():
    betas = np.linspace(BETA_1, BETA_T, T, dtype=np.float64)
    return np.cumprod(1.0 - betas)


def _host_constants(W, b, temb, t):
    """Coefficient blob [B, 128, 512] fp8 + per-batch noise channel shift c.

    Per batch, per partition p (k-row): bytes [0:128) Le, [128:256) Lo,
    [256:384) -I, [384:512) NzB = s_omab*W pattern; columns are the matmul
    M index (4 outch x 32 rowpairs).
    """
    W = np.asarray(W, dtype=np.float64)
    b = np.asarray(b, dtype=np.float64)
    temb = np.asarray(temb, dtype=np.float64)
    t = np.asarray(t).astype(np.int64)

    alphas_bar = _schedule()
    s_ab = np.sqrt(alphas_bar[t])
    s_omab = np.sqrt(1.0 - alphas_bar[t])

    B = t.shape[0]
    coef = np.zeros((B, 128, 4, 128), dtype=np.float64)
    cshift = np.zeros((B, 4), dtype=np.float64)

    i_idx = np.arange(PAIRS)
    eye4 = np.eye(4)
    for bi in range(B):
        KA = np.einsum("ok,krc->orc", W[:, 0:4], _HAAR) * s_ab[bi]
        KB = np.einsum("ok,krc->orc", W[:, 4:8] - s_ab[bi] * W[:, 0:4], _HAAR)
        beta = b + temb[t[bi]]
        cshift[bi] = np.linalg.solve(s_omab[bi] * W[:, 0:4] - eye4, beta)
        for o in range(4):
            m = o * PAIRS + i_idx
            for r in range(2):
                coef[bi, 2 * i_idx + r, 0, m] = KA[o, r, 0]
                coef[bi, 2 * i_idx + r, 1, m] = KA[o, r, 1]
                coef[bi, 64 + 2 * i_idx + r, 0, m] = KB[o, r, 0]
                coef[bi, 64 + 2 * i_idx + r, 1, m] = KB[o, r, 1]
            for c in range(4):
                coef[bi, c * PAIRS + i_idx, 2, m] = -1.0 if o == c else 0.0
                coef[bi, c * PAIRS + i_idx, 3, m] = s_omab[bi] * W[o, c]

    return coef.astype(NPFP8).reshape(B, 128, 512), cshift


def _shuffle_x0(x0_shard):
    """[B,2,512,512] fp32 -> [B, 128, 4, 2, 2, 256] fp8; partition p = ch*64 +
    (row%64); free = (slabpair, parity, slab01, colpair) so one DoubleRow
    matmul covers a 512-col PSUM bank (2 slabs)."""
    B = x0_shard.shape[0]
    v = x0_shard.reshape(B, 2, 4, 2, 64, WO, 2)
    # (B, ch, pair, slab01, row64, colpair, parity)
    v = v.transpose(0, 1, 4, 2, 6, 3, 5)
    # -> (B, ch, row64, pair, parity, slab01, colpair)
    return np.ascontiguousarray(v.reshape(B, 128, 4, 1024)).astype(NPFP8)


def _shuffle_nz(nz_shard, cshift):
    """[B,4,256,256] fp32 + shift -> [B, 128, 8, 256] fp8;
    partition p = ch*32 + pair%32."""
    B = nz_shard.shape[0]
    v = nz_shard + cshift[:, :, None, None]
    v = v.reshape(B, 4, N_SLABS, PAIRS, WO)
    v = v.transpose(0, 1, 3, 2, 4)
    return np.ascontiguousarray(v.reshape(B, 128, N_SLABS, WO)).astype(NPFP8)


def build_nc(debug=False, n_ramp=N_RAMP):
    """Build the per-core Bass program (same program on all 8 cores)."""
    nc = bacc.Bacc("TRN2", target_bir_lowering=False, debug=debug)

    # uint8 on the wire: fp8 arrays take a flaky path through the PJRT/axon
    # upload, a same-byte uint8 view does not; APs bitcast to fp8 on device
    U8 = mybir.dt.uint8
    data_d = nc.declare_dram_parameter("data", [B_PER, 128, BLOB], U8, isOutput=False)
    out_d = nc.declare_dram_parameter("out", [128, N_OUT], F32, isOutput=True)

    DR = mybir.MatmulPerfMode.DoubleRow

    with tile.TileContext(nc) as tc:
        with (
            tc.tile_pool(name="sb", bufs=1) as sb_pool,
            tc.tile_pool(name="psum", bufs=2, space="PSUM") as psum_pool,
        ):
            partials = sb_pool.tile([128, N_OUT], F32, tag="partials")
            warm = sb_pool.tile([128, 1], F32, tag="warm")
            # warm up the Square activation table during the DMA ramp so the
            # ~1.5us ACT_TABLE_LOAD is off the critical path
            nc.gpsimd.memset(warm[:], 0.0)
            nc.scalar.activation(
                warm[:], warm[:], mybir.ActivationFunctionType.Square
            )

            # tiny transfers warm both HWDGE rings: the first trigger on a
            # cold ring takes ~2.1us to produce packets, so pay that on a
            # throwaway 128-byte read while the real descriptors generate
            rw0 = sb_pool.tile([128, 1], U8, tag="rw0")
            rw1 = sb_pool.tile([128, 1], U8, tag="rw1")
            nc.sync.dma_start(rw0[:], data_d[0, :, 0:1])
            nc.scalar.dma_start(rw1[:], data_d[0, :, 1:2])

            # all four blobs resident (bufs=4): every DMA trigger fires at
            # t~0 with no WAR hazards. Two HWDGE rings (sync: even batches,
            # scalar: odd), each blob in two halves so batch 0's matmuls
            # start ~2us earlier. The scalar ring's descriptor generation
            # (ring backpressure pauses it mid-stream) finishes before the
            # first square's semaphore wait needs the scalar sequencer, and
            # the Square table load was issued before it.
            tiles = []
            for b in range(B_PER):
                dt = sb_pool.tile([128, BLOB], U8, tag="blob", bufs=4, name="dt")
                eng = nc.sync if b % 2 == 0 else nc.scalar
                eng.dma_start(dt[:, 0:OFF_XB], data_d[b, :, 0:OFF_XB])
                eng.dma_start(dt[:, OFF_XB:BLOB], data_d[b, :, OFF_XB:BLOB])
                tiles.append(dt)

            # PE DVFS pre-ramp: junk matmuls with no DMA deps keep the PE
            # busy from kernel start until batch 0's data lands, so real
            # matmuls run at the ramped clock sooner. They read the
            # load-time-initialized const-1.0 tensor through stride-0
            # broadcast APs (no runtime memset dependency) and write into
            # batch 0's PSUM tile, which the real matmuls then overwrite
            # (start=True).
            ps0 = psum_pool.tile([128, 2048], F32, tag="ps")
            cone = nc.const_aps.tensor(1.0, (128, 1), F32)
            junk_st = cone.bitcast(FP8)[:, 0:1].unsqueeze(1).broadcast_to(
                [128, 2, 128]
            )
            junk_mv = cone.bitcast(FP8)[:, 0:1].unsqueeze(1).broadcast_to(
                [128, 2, 512]
            )
            for _ in range(n_ramp):
                nc.tensor.matmul(
                    ps0[:, 0:512], junk_st, junk_mv,
                    start=True, stop=True, perf_mode=DR,
                )

            sq = sb_pool.tile([128, 2048], F32, tag="sq")

            acc_col = 0
            for b in range(B_PER):
                dt = tiles[b]
                lelo = (
                    dt[:, OFF_COEF : OFF_COEF + 256]
                    .bitcast(FP8)
                    .rearrange("p (k m) -> p k m", k=2)
                )
                nzmat = (
                    dt[:, OFF_COEF + 256 : OFF_COEF + 512]
                    .bitcast(FP8)
                    .rearrange("p (k m) -> p k m", k=2)
                )

                ps = ps0 if b == 0 else psum_pool.tile([128, 2048], F32, tag="ps")
                for p in range(4):  # slab-pairs; one 512-col PSUM bank each
                    if p < 2:
                        xoff = OFF_XA + p * 1024
                        noff = OFF_NZA + p * 512
                    else:
                        xoff = OFF_XB + (p - 2) * 1024
                        noff = OFF_NZB + (p - 2) * 512
                    xg = (
                        dt[:, xoff : xoff + 1024]
                        .bitcast(FP8)
                        .rearrange("p (k n) -> p k n", k=2)
                    )
                    nzg = (
                        dt[:, noff : noff + 512]
                        .bitcast(FP8)
                        .unsqueeze(1)
                        .broadcast_to([128, 2, 512])
                    )
                    seg = ps[:, p * 512 : (p + 1) * 512]
                    nc.tensor.matmul(seg, lelo, xg, start=True, stop=False,
                                     perf_mode=DR)
                    nc.tensor.matmul(seg, nzmat, nzg, start=False, stop=True,
                                     perf_mode=DR)

                # one Square+accum per batch (bias was absorbed into the
                # host-side noise shift); early batches are split so the
                # ACT stream starts as soon as the first PSUM banks are
                # written (subtile deps), and the last batch so only a
                # 384-col piece trails the final matmul
                if b == 0:
                    pieces = [(0, 1024), (1024, 2048)]
                elif b < B_PER - 1:
                    pieces = [(0, 2048)]
                else:
                    pieces = [(0, SPLIT), (SPLIT, 2048)]
                for lo, hi in pieces:
                    nc.scalar.activation(
                        sq[:, lo:hi],
                        ps[:, lo:hi],
                        mybir.ActivationFunctionType.Square,
                        scale=1.0,
                        accum_out=partials[:, acc_col : acc_col + 1],
                    )
                    acc_col += 1

            nc.sync.dma_start(out_d[:], partials[:])

    nc.compile()
    return nc


_NC_CACHE = None


def _get_nc():
    global _NC_CACHE
    if _NC_CACHE is None:
        _NC_CACHE = build_nc()
    return _NC_CACHE


def make_in_maps(x_0, noise, W, b, temb, t):
    x_0 = np.asarray(x_0, dtype=np.float32)
    noise = np.asarray(noise, dtype=np.float32)
    coef, cshift = _host_constants(W, b, temb, t)
    xs = _shuffle_x0(x_0)            # [B, 128, 4, 1024] fp8
    ns = _shuffle_nz(noise, cshift)  # [B, 128, 8, 256] fp8

    in_maps = []
    for c in range(N_CORES):
        s = slice(c * B_PER, (c + 1) * B_PER)
        xc, nc_, cc = xs[s], ns[s], coef[s]
        blob = np.empty((B_PER, 128, BLOB), dtype=np.uint8)
        bv = blob.view(NPFP8)
        bv[:, :, OFF_COEF:OFF_XA] = cc
        bv[:, :, OFF_XA:OFF_NZA] = xc[:, :, :2].reshape(B_PER, 128, 2048)
        bv[:, :, OFF_NZA:OFF_XB] = nc_[:, :, :4].reshape(B_PER, 128, 1024)
        bv[:, :, OFF_XB:OFF_NZB] = xc[:, :, 2:].reshape(B_PER, 128, 2048)
        bv[:, :, OFF_NZB:BLOB] = nc_[:, :, 4:].reshape(B_PER, 128, 1024)
        in_maps.append({"data": blob})
    return in_maps


def kernel(x_0, noise, W, b, temb, t, **_ignored):
    nc = _get_nc()
    in_maps = make_in_maps(x_0, noise, W, b, temb, t)
    res = run_bass_kernel_spmd(nc, in_maps, list(range(N_CORES)))
    total = 0.0
    for c in range(N_CORES):
        total += float(res.results[c]["out"].astype(np.float64).sum())
    return np.float32(total)


# revision 26
# speedup vs baseline: 1.0119x; 1.0119x over previous
"""Trainium2 Bass kernel for the D3CG trainer-loss problem.

Computes, for full inputs:
    loss = sum((eps_theta - noise)**2)
where eps_theta is a 1x1-conv surrogate denoiser applied to
[d_t, cbct_coeffs] built from Haar DWT coefficients of x_0's two channels.

Strategy (pure data parallel over batch, 4 batches per core on 8 cores):
Everything before the square is linear in (x_0, noise) per output pixel, with
per-batch scalar coefficients, so each 64-image-row slab reduces to matmuls
against small host-precomputed coefficient matrices:

  r = Le^T @ x_even + Lo^T @ x_odd + (-I + NzB)^T @ noise'

laid out [4 outch x 32 rowpairs, 256 cols] in PSUM, where noise' = noise + c
with the per-batch channel shift c solving (s_omab*W1 - I) c = b + temb[t]:
the constant shift reproduces the bias exactly through the noise mixing, so
no bias add is needed anywhere on device. One ScalarE Square per batch with
accum_out reduces each [128, 2048] PSUM tile to per-partition partial sums
(the last batch in two pieces so only a 512-col piece trails the final
matmul instead of a full 2048-col pass).

All data streams in fp8e4m3 (host casts; final loss rel err ~2.5e-3 vs the
2e-2 gate) and both matmuls per slab-pair bank run in DoubleRow perf mode.
A chain of dummy matmuls on junk SBUF data runs while the input DMAs ramp,
pushing the PE DVFS clock up before the first real matmul so real matmuls
run near the fast steady-state rate from the start.

DMA: one uint8 blob per batch [128, 6656] (coefs | x/nz slabs 0-3 | x/nz
slabs 4-7), bitcast to fp8 on device, issued as two transfers on the two
HWDGE dynamic rings (sync + scalar engines, even/odd batches) so all
triggers fire right after the NEFF prologue and batch 0 computes while
later batches stream. Total HBM traffic ~3.3 MiB/core.
"""

import sys

if "/opt/trn_rl_repo" not in sys.path:
    sys.path.insert(0, "/opt/trn_rl_repo")

import numpy as np
import ml_dtypes

import concourse.bass as bass  # noqa: F401
import concourse.mybir as mybir
import concourse.tile as tile
from concourse import bacc
from concourse.bass_utils import run_bass_kernel_spmd

T = 1000
BETA_1 = 1e-4
BETA_T = 0.02

N_CORES = 8
B_TOTAL = 32
B_PER = B_TOTAL // N_CORES  # 4 batches per core
H = 512
Wd = 512
N_SLABS = 8                 # 64-image-row slabs per image
PAIRS = 32                  # output rows per slab
WO = 256                    # output cols

F32 = mybir.dt.float32
FP8 = mybir.dt.float8e4
NPFP8 = ml_dtypes.float8_e4m3

# per-partition byte offsets inside the per-batch fp8 blob [128, 6656]
OFF_COEF = 0        # LeLo [2,128] + NzMat [2,128] = 512 B
OFF_XA = 512        # x slab-pairs 0-1: [2 pair, 2 parity, 2 slab, 256] = 2048 B
OFF_NZA = 2560      # nz slabs 0-3: [4, 256] = 1024 B
OFF_XB = 3584       # x slab-pairs 2-3
OFF_NZB = 5632      # nz slabs 4-7
BLOB = 6656

# the first batch's square runs as two 1024-col pieces (starts as soon as
# its first two PSUM banks are written), the last batch as 1536+512 so
# only a 512-col piece trails the final matmul. (A DVE-assisted split was
# tried and reverted: DVE fp32 runs at ~1.3ns/elem and its PSUM reads
# stalled the matmul pipeline, dropping the PE clock.)
SPLIT = 1536
N_OUT = B_PER + 2

# dummy matmuls to ramp the PE clock while input DMAs stream
N_RAMP = 8

# Haar 2x2 analysis kernels for [cA, cH, cV, cD] as functions of the block
# [[a, b], [c, d]] = [[x[2i,2j], x[2i,2j+1]], [x[2i+1,2j], x[2i+1,2j+1]]].
_HAAR = 0.5 * np.array(
    [
        [[1.0, 1.0], [1.0, 1.0]],    # cA
        [[1.0, 1.0], [-1.0, -1.0]],  # cH (detail axis0)
        [[1.0, -1.0], [1.0, -1.0]],  # cV (detail axis1)
        [[1.0, -1.0], [-1.0, 1.0]],  # cD
    ],
    dtype=np.float64,
)


def _schedule():
    betas = np.linspace(BETA_1, BETA_T, T, dtype=np.float64)
    return np.cumprod(1.0 - betas)


def _host_constants(W, b, temb, t):
    """Coefficient blob [B, 128, 512] fp8 + per-batch noise channel shift c.

    Per batch, per partition p (k-row): bytes [0:128) Le, [128:256) Lo,
    [256:384) -I, [384:512) NzB = s_omab*W pattern; columns are the matmul
    M index (4 outch x 32 rowpairs).
    """
    W = np.asarray(W, dtype=np.float64)
    b = np.asarray(b, dtype=np.float64)
    temb = np.asarray(temb, dtype=np.float64)
    t = np.asarray(t).astype(np.int64)

    alphas_bar = _schedule()
    s_ab = np.sqrt(alphas_bar[t])
    s_omab = np.sqrt(1.0 - alphas_bar[t])

    B = t.shape[0]
    coef = np.zeros((B, 128, 4, 128), dtype=np.float64)
    cshift = np.zeros((B, 4), dtype=np.float64)

    i_idx = np.arange(PAIRS)
    eye4 = np.eye(4)
    for bi in range(B):
        KA = np.einsum("ok,krc->orc", W[:, 0:4], _HAAR) * s_ab[bi]
        KB = np.einsum("ok,krc->orc", W[:, 4:8] - s_ab[bi] * W[:, 0:4], _HAAR)
        beta = b + temb[t[bi]]
        cshift[bi] = np.linalg.solve(s_omab[bi] * W[:, 0:4] - eye4, beta)
        for o in range(4):
            m = o * PAIRS + i_idx
            for r in range(2):
                coef[bi, 2 * i_idx + r, 0, m] = KA[o, r, 0]
                coef[bi, 2 * i_idx + r, 1, m] = KA[o, r, 1]
                coef[bi, 64 + 2 * i_idx + r, 0, m] = KB[o, r, 0]
                coef[bi, 64 + 2 * i_idx + r, 1, m] = KB[o, r, 1]
            for c in range(4):
                coef[bi, c * PAIRS + i_idx, 2, m] = -1.0 if o == c else 0.0
                coef[bi, c * PAIRS + i_idx, 3, m] = s_omab[bi] * W[o, c]

    return coef.astype(NPFP8).reshape(B, 128, 512), cshift


def _shuffle_x0(x0_shard):
    """[B,2,512,512] fp32 -> [B, 128, 4, 2, 2, 256] fp8; partition p = ch*64 +
    (row%64); free = (slabpair, parity, slab01, colpair) so one DoubleRow
    matmul covers a 512-col PSUM bank (2 slabs)."""
    B = x0_shard.shape[0]
    v = x0_shard.reshape(B, 2, 4, 2, 64, WO, 2)
    # (B, ch, pair, slab01, row64, colpair, parity)
    v = v.transpose(0, 1, 4, 2, 6, 3, 5)
    # -> (B, ch, row64, pair, parity, slab01, colpair)
    return np.ascontiguousarray(v.reshape(B, 128, 4, 1024)).astype(NPFP8)


def _shuffle_nz(nz_shard, cshift):
    """[B,4,256,256] fp32 + shift -> [B, 128, 8, 256] fp8;
    partition p = ch*32 + pair%32."""
    B = nz_shard.shape[0]
    v = nz_shard + cshift[:, :, None, None]
    v = v.reshape(B, 4, N_SLABS, PAIRS, WO)
    v = v.transpose(0, 1, 3, 2, 4)
    return np.ascontiguousarray(v.reshape(B, 128, N_SLABS, WO)).astype(NPFP8)


def build_nc(debug=False, n_ramp=N_RAMP):
    """Build the per-core Bass program (same program on all 8 cores)."""
    nc = bacc.Bacc("TRN2", target_bir_lowering=False, debug=debug)

    # uint8 on the wire: fp8 arrays take a flaky path through the PJRT/axon
    # upload, a same-byte uint8 view does not; APs bitcast to fp8 on device
    U8 = mybir.dt.uint8
    data_d = nc.declare_dram_parameter("data", [B_PER, 128, BLOB], U8, isOutput=False)
    out_d = nc.declare_dram_parameter("out", [128, N_OUT], F32, isOutput=True)

    DR = mybir.MatmulPerfMode.DoubleRow

    with tile.TileContext(nc) as tc:
        with (
            tc.tile_pool(name="sb", bufs=1) as sb_pool,
            tc.tile_pool(name="psum", bufs=2, space="PSUM") as psum_pool,
        ):
            partials = sb_pool.tile([128, N_OUT], F32, tag="partials")
            warm = sb_pool.tile([128, 1], F32, tag="warm")
            # warm up the Square activation table during the DMA ramp so the
            # ~1.5us ACT_TABLE_LOAD is off the critical path
            nc.gpsimd.memset(warm[:], 0.0)
            nc.scalar.activation(
                warm[:], warm[:], mybir.ActivationFunctionType.Square
            )

            # a tiny transfer warms the sync HWDGE ring: the first trigger
            # on a cold ring takes ~2.1us to produce packets, so pay that
            # on a throwaway 128-byte read while the real descriptors
            # generate
            rw0 = sb_pool.tile([128, 1], U8, tag="rw0")
            nc.sync.dma_start(rw0[:], data_d[0, :, 0:1])

            # all four blobs resident (bufs=4): every DMA trigger fires at
            # t~0 with no WAR hazards. One full-blob transfer per batch,
            # all on the sync HWDGE ring: a second ring steals engine
            # bandwidth from batch 0's transfer exactly when it is the
            # critical dependency, and 6656-byte descriptors waste fewer
            # ring turnaround bubbles than halves. The scalar sequencer
            # stays free for the squares.
            tiles = []
            for b in range(B_PER):
                dt = sb_pool.tile([128, BLOB], U8, tag="blob", bufs=4, name="dt")
                nc.sync.dma_start(dt[:], data_d[b, :, :])
                tiles.append(dt)

            # PE DVFS pre-ramp: junk matmuls with no DMA deps keep the PE
            # busy from kernel start until batch 0's data lands, so real
            # matmuls run at the ramped clock sooner. They read the
            # load-time-initialized const-1.0 tensor through stride-0
            # broadcast APs (no runtime memset dependency) and write into
            # batch 0's PSUM tile, which the real matmuls then overwrite
            # (start=True).
            ps0 = psum_pool.tile([128, 2048], F32, tag="ps")
            cone = nc.const_aps.tensor(1.0, (128, 1), F32)
            junk_st = cone.bitcast(FP8)[:, 0:1].unsqueeze(1).broadcast_to(
                [128, 2, 128]
            )
            junk_mv = cone.bitcast(FP8)[:, 0:1].unsqueeze(1).broadcast_to(
                [128, 2, 512]
            )
            for _ in range(n_ramp):
                nc.tensor.matmul(
                    ps0[:, 0:512], junk_st, junk_mv,
                    start=True, stop=True, perf_mode=DR,
                )

            sq = sb_pool.tile([128, 2048], F32, tag="sq")

            acc_col = 0
            for b in range(B_PER):
                dt = tiles[b]
                lelo = (
                    dt[:, OFF_COEF : OFF_COEF + 256]
                    .bitcast(FP8)
                    .rearrange("p (k m) -> p k m", k=2)
                )
                nzmat = (
                    dt[:, OFF_COEF + 256 : OFF_COEF + 512]
                    .bitcast(FP8)
                    .rearrange("p (k m) -> p k m", k=2)
                )

                ps = ps0 if b == 0 else psum_pool.tile([128, 2048], F32, tag="ps")
                for p in range(4):  # slab-pairs; one 512-col PSUM bank each
                    if p < 2:
                        xoff = OFF_XA + p * 1024
                        noff = OFF_NZA + p * 512
                    else:
                        xoff = OFF_XB + (p - 2) * 1024
                        noff = OFF_NZB + (p - 2) * 512
                    xg = (
                        dt[:, xoff : xoff + 1024]
                        .bitcast(FP8)
                        .rearrange("p (k n) -> p k n", k=2)
                    )
                    nzg = (
                        dt[:, noff : noff + 512]
                        .bitcast(FP8)
                        .unsqueeze(1)
                        .broadcast_to([128, 2, 512])
                    )
                    seg = ps[:, p * 512 : (p + 1) * 512]
                    nc.tensor.matmul(seg, lelo, xg, start=True, stop=False,
                                     perf_mode=DR)
                    nc.tensor.matmul(seg, nzmat, nzg, start=False, stop=True,
                                     perf_mode=DR)

                # one Square+accum per batch (bias was absorbed into the
                # host-side noise shift); early batches are split so the
                # ACT stream starts as soon as the first PSUM banks are
                # written (subtile deps), and the last batch so only a
                # 384-col piece trails the final matmul
                if b == 0:
                    pieces = [(0, 1024), (1024, 2048)]
                elif b < B_PER - 1:
                    pieces = [(0, 2048)]
                else:
                    pieces = [(0, SPLIT), (SPLIT, 2048)]
                for lo, hi in pieces:
                    nc.scalar.activation(
                        sq[:, lo:hi],
                        ps[:, lo:hi],
                        mybir.ActivationFunctionType.Square,
                        scale=1.0,
                        accum_out=partials[:, acc_col : acc_col + 1],
                    )
                    acc_col += 1

            nc.sync.dma_start(out_d[:], partials[:])

    nc.compile()
    return nc


_NC_CACHE = None


def _get_nc():
    global _NC_CACHE
    if _NC_CACHE is None:
        _NC_CACHE = build_nc()
    return _NC_CACHE


def make_in_maps(x_0, noise, W, b, temb, t):
    x_0 = np.asarray(x_0, dtype=np.float32)
    noise = np.asarray(noise, dtype=np.float32)
    coef, cshift = _host_constants(W, b, temb, t)
    xs = _shuffle_x0(x_0)            # [B, 128, 4, 1024] fp8
    ns = _shuffle_nz(noise, cshift)  # [B, 128, 8, 256] fp8

    in_maps = []
    for c in range(N_CORES):
        s = slice(c * B_PER, (c + 1) * B_PER)
        xc, nc_, cc = xs[s], ns[s], coef[s]
        blob = np.empty((B_PER, 128, BLOB), dtype=np.uint8)
        bv = blob.view(NPFP8)
        bv[:, :, OFF_COEF:OFF_XA] = cc
        bv[:, :, OFF_XA:OFF_NZA] = xc[:, :, :2].reshape(B_PER, 128, 2048)
        bv[:, :, OFF_NZA:OFF_XB] = nc_[:, :, :4].reshape(B_PER, 128, 1024)
        bv[:, :, OFF_XB:OFF_NZB] = xc[:, :, 2:].reshape(B_PER, 128, 2048)
        bv[:, :, OFF_NZB:BLOB] = nc_[:, :, 4:].reshape(B_PER, 128, 1024)
        in_maps.append({"data": blob})
    return in_maps


def kernel(x_0, noise, W, b, temb, t, **_ignored):
    nc = _get_nc()
    in_maps = make_in_maps(x_0, noise, W, b, temb, t)
    res = run_bass_kernel_spmd(nc, in_maps, list(range(N_CORES)))
    total = 0.0
    for c in range(N_CORES):
        total += float(res.results[c]["out"].astype(np.float64).sum())
    return np.float32(total)
